# revision 14
# baseline (speedup 1.0000x reference)
"""Head-sharded (tensor-parallel) causal self-attention with RoPE and T5-style
relative position bias, running SPMD on 8 Trainium2 NeuronCores.

Sharding: heads are split 2-per-core (H=16 over 8 cores). Each core computes
its heads' QKV projection (full token range, fp32r matmuls), RoPE, causal
attention with the rel-pos bias folded into a Toeplitz "bias strip", and a
partial output projection against its slice of w_o's columns. The host sums
the 8 partial outputs.

Attention uses the S^T layout (keys on partitions, queries on the free dim):
  S^T[k, q] = sum_d k'[d,k] q'[d,q]      (PE, contraction over d=128)
  P = exp(S^T/sqrt(D) + bias)            (DVE bias add + ACT exp)
  y^T[d, q] += V^T-free matmul           (PE, lhsT = V token-major)
  l[q] += ones^T @ P                     (PE)
Scores here are O(+-5), so softmax runs without the running-max rescale.
The causal mask and the rel-pos bias are both baked into one per-head
[128, T] strip (bias of block-diagonal offset delta depends only on q-k).
"""

import json
import numpy as np
from contextlib import ExitStack

import concourse.bass as bass
import concourse.bass2jax as _b2j
import concourse.bass_utils as _bu
import concourse.mybir as mybir
import concourse.tile as tile
from concourse.bass_utils import run_bass_kernel_spmd
from concourse.vector_clock import ScopedClock

import ml_dtypes

# ----------------------------------------------------------------------------
# Walrus compatibility: this image's walrus codegen accepts at most ONE sync
# wait per instruction (setupSyncWait "Too many sync wait commands"). Tile
# freely emits multi-wait instructions, so rewrite the BIR before compiling:
# excess waits move onto single-wait NoOps inserted just before the
# instruction on the same engine (same-engine program order makes this
# semantically identical).
# ----------------------------------------------------------------------------

_ws_counter = [0]


def _split_multi_waits(obj):
    if isinstance(obj, dict):
        insts = obj.get("instructions")
        if isinstance(insts, list):
            newl = []
            for ins in insts:
                si = ins.get("sync_info") if isinstance(ins, dict) else None
                waits = (si or {}).get("on_wait") or []
                if len(waits) > 1:
                    for w in waits[:-1]:
                        _ws_counter[0] += 1
                        newl.append({
                            "name": f"I-wsplit-{_ws_counter[0]}",
                            "opcode": "NoOp",
                            "engine": ins["engine"],
                            "ins": [], "outs": [],
                            "debug": ins.get("debug", 0),
                            "sync_info": {"on_wait": [w], "on_update": []},
                        })
                    si["on_wait"] = waits[-1:]
                newl.append(ins)
            obj["instructions"] = newl
        for v in obj.values():
            _split_multi_waits(v)
    elif isinstance(obj, list):
        for v in obj:
            _split_multi_waits(v)


_orig_compile_bir_kernel = _bu.compile_bir_kernel


def _compile_bir_kernel_split_waits(bir_json, *args, **kwargs):
    if isinstance(bir_json, (bytes, bytearray)):
        bir_json = bytes(bir_json)
    bir = json.loads(bir_json)
    _split_multi_waits(bir)
    return _orig_compile_bir_kernel(json.dumps(bir).encode(), *args, **kwargs)


if _bu.compile_bir_kernel is _orig_compile_bir_kernel:
    _bu.compile_bir_kernel = _compile_bir_kernel_split_waits
    _b2j.compile_bir_kernel = _compile_bir_kernel_split_waits


def _drain_and_barrier_split_waits(self, tick_clock, wait_clock):
    """TileContext tail-drain fix for this walrus build: the CoreV3 codegen
    rejects >1 sync-wait on the drain ("Too many sync wait commands"), so park
    the global-clock waits on one NOP each before a bare drain."""
    nop1 = self.nc.sync.nop(nofuse=True)
    wait_clock.add_sem_waits(nop1.ins, ScopedClock({None: tick_clock.global_clock}))
    si = nop1.ins.sync_info
    waits = list(si.on_wait) if si else []
    if len(waits) > 1:
        si.on_wait[:] = waits[:1]
        for w in waits[1:]:
            extra = self.nc.sync.nop(nofuse=True)
            extra.ins.sync_info = mybir.SyncInfo(on_wait=[w], on_update=[])
    self.nc.sync.drain()
    self.nc.all_engine_barrier()
    assert self.sems is not None
    popped = self.nc._tile_sem_poison_stack.pop()
    assert popped is self._sem_poison
    self.nc.clear_and_free_semaphores(list(self.sems.allocated().values()))
    self.nc.all_engine_barrier()


tile.TileContext._drain_and_barrier = _drain_and_barrier_split_waits

# Problem shapes (hardcoded for this task).
B, T, C, H, D = 2, 2048, 2048, 16, 128
NUM_BUCKETS, MAX_DIST = 32, 128
NCORES = 8
HPC = H // NCORES          # heads per core = 2
BT = B * T                 # 4096 tokens
CCH = C // 128             # 16 contraction chunks over C
SLAB = 256                 # qkv token slab width (moving dim, >=256 for f32r rate)
QS = 512                   # attention query subchunk
NQS = T // QS              # 4
NKC = T // 128             # 16 key chunks per sequence
NEG = np.float32(-1.0e30)
SCALE = float(1.0 / np.sqrt(D))

f32 = mybir.dt.float32
f32r = mybir.dt.float32r
bf16 = mybir.dt.bfloat16
AF = mybir.ActivationFunctionType
BF16NP = ml_dtypes.bfloat16

# dtype knobs (accuracy / SBUF tradeoffs)
QK_DT = f32r     # q/k staging + S matmul (f32r = full PE rate, ~fp32 storage)
V_DT = bf16      # v staging, P tiles, PV matmul dtype
YN_DT = bf16     # normalized attention out + w_o operand dtype
TBL_DT = bf16    # bias strips + rope cos/sin tables


def _np_of(dt):
    return {f32: np.float32, f32r: np.float32, bf16: BF16NP}[dt]


def _mm(ap):
    """Bitcast f32 matmul operands to f32r (full PE rate at N>=256)."""
    return ap.bitcast(f32r) if ap.dtype == f32 else ap


# ----------------------------------------------------------------------------
# Host-side tables (shape-only, cached)
# ----------------------------------------------------------------------------

def _rel_bucket(n):
    """T5 unidirectional bucket of past-distance n (>=0), mirrors reference."""
    me = NUM_BUCKETS // 2
    nf = np.maximum(n.astype(np.float32), np.float32(1.0))
    val = np.log(nf / me).astype(np.float32) / np.float32(np.log(MAX_DIST / me))
    val = (val * np.float32(NUM_BUCKETS - me)).astype(np.float32)
    val_large = me + val.astype(np.int32)
    val_large = np.minimum(val_large, NUM_BUCKETS - 1)
    return np.where(n < me, n, val_large)


_TBL_CACHE = {}


def _bucket_strip():
    """[128, T] int bucket ids for strip position (q' - kk); -1 where masked."""
    if "bstrip" not in _TBL_CACHE:
        kk = np.arange(128)[:, None]
        qp = np.arange(T)[None, :]
        n = qp - kk                              # distance into the past
        buck = _rel_bucket(np.maximum(n, 0))
        _TBL_CACHE["bstrip"] = (buck, n < 0)
    return _TBL_CACHE["bstrip"]


def _rope_tables():
    if "rope" not in _TBL_CACHE:
        half = np.arange(0, D, 2, dtype=np.float32) / np.float32(D)
        inv_freq = (1.0 / (10000.0 ** half)).astype(np.float32)
        freqs = np.arange(T, dtype=np.float32)[:, None] * inv_freq[None, :]
        emb = np.concatenate([freqs, freqs], axis=1).astype(np.float32)  # [T, D]
        cosT = np.ascontiguousarray(np.cos(emb).T)     # [D, T]
        sinT = np.ascontiguousarray(np.sin(emb).T)
        sinT[: D // 2] = -sinT[: D // 2]               # fold rotate_half sign
        _TBL_CACHE["rope"] = (cosT, sinT)
    return _TBL_CACHE["rope"]


# ----------------------------------------------------------------------------
# Device program
# ----------------------------------------------------------------------------

def _emit(tc, xT, wqT, wkT, wvT, woT, biasS, cosT, sinTs, out):
    nc = tc.nc
    qknp = QK_DT

    xTv = xT.rearrange("(i p) n -> p i n", p=128)       # [128, CCH, BT]
    wqv = wqT.rearrange("(i p) f -> p i f", p=128)      # [128, CCH, HPC*D]
    wkv = wkT.rearrange("(i p) f -> p i f", p=128)
    wvv = wvT.rearrange("(i p) f -> p i f", p=128)
    wov = woT.rearrange("h d n -> d h n")               # [128, HPC, C]
    outv = out.rearrange("(nt p) c -> nt p c", p=128)   # [BT/128, 128, C]

    with ExitStack() as ctx:
        ep = ctx.enter_context

        # SBUF pools
        wpool = ep(tc.tile_pool(name="weights", bufs=1))
        xpool = ep(tc.tile_pool(name="xslab", bufs=2))
        qkvp = ep(tc.tile_pool(name="qkv", bufs=1))
        shufp = ep(tc.tile_pool(name="shuf", bufs=2))
        t2p = ep(tc.tile_pool(name="t2", bufs=1))
        prep = ep(tc.tile_pool(name="pre", bufs=2))
        ptp = ep(tc.tile_pool(name="pt", bufs=4))
        ynp = ep(tc.tile_pool(name="yn", bufs=1))
        ostp = ep(tc.tile_pool(name="ost", bufs=2))
        smallp = ep(tc.tile_pool(name="small", bufs=2))
        bcp = ep(tc.tile_pool(name="bcp", bufs=1))

        # PSUM pools (8 banks total budget)
        psA = ep(tc.tile_pool(name="psA", bufs=3, space="PSUM"))   # qk-proj + S
        psB = ep(tc.tile_pool(name="psB", bufs=2, space="PSUM"))   # v-proj + oproj
        psY = ep(tc.tile_pool(name="psY", bufs=1, space="PSUM"))   # y^T accum
        psL = ep(tc.tile_pool(name="psL", bufs=1, space="PSUM"))   # l accum
        psBC = ep(tc.tile_pool(name="psBC", bufs=1, space="PSUM"))  # 1/l broadcast

        # Resident weights/tables
        wq_s = wpool.tile([128, CCH, HPC * D], f32r, tag="wq")
        nc.sync.dma_start(wq_s[:], wqv[:])
        wk_s = wpool.tile([128, CCH, HPC * D], f32r, tag="wk")
        nc.sync.dma_start(wk_s[:], wkv[:])
        wv_s = wpool.tile([128, CCH, HPC * D], f32r, tag="wv")
        nc.sync.dma_start(wv_s[:], wvv[:])
        wo_s = wpool.tile([128, HPC, C], YN_DT, tag="wo")
        nc.sync.dma_start(wo_s[:], wov[:])
        bias_s = wpool.tile([128, HPC, T], TBL_DT, tag="bias")
        nc.sync.dma_start(bias_s[:], biasS[:])
        cos_s = wpool.tile([128, T], TBL_DT, tag="cos")
        nc.sync.dma_start(cos_s[:], cosT[:])
        sin_s = wpool.tile([128, T], TBL_DT, tag="sin")
        nc.sync.dma_start(sin_s[:], sinTs[:])
        ones_col = wpool.tile([128, 1], V_DT, tag="ones_col")
        nc.gpsimd.memset(ones_col[:], 1.0)
        ones_row = wpool.tile([1, 128], f32, tag="ones_row")
        nc.gpsimd.memset(ones_row[:], 1.0)

        def qkv_phase(b):
            q_raw = qkvp.tile([128, HPC, T], qknp, tag="qraw")
            k_raw = qkvp.tile([128, HPC, T], qknp, tag="kraw")
            v_tm = qkvp.tile([128, NKC, HPC * D], V_DT, tag="vtm")
            for sl in range(T // SLAB):
                t0 = b * T + sl * SLAB
                slab = xpool.tile([128, CCH, SLAB], f32r, tag="slab")
                nc.sync.dma_start(slab[:], xTv[:, :, t0:t0 + SLAB])
                # q/k feature-major: out [feat 128, tok SLAB]
                for wt, dst, nm in ((wq_s, q_raw, "q"), (wk_s, k_raw, "k")):
                    for h in range(HPC):
                        ps = psA.tile([128, SLAB], f32, tag="psA")
                        for ci in range(CCH):
                            nc.tensor.matmul(
                                ps[:],
                                lhsT=_mm(wt[:, ci, h * D:(h + 1) * D]),
                                rhs=_mm(slab[:, ci, :]),
                                start=(ci == 0), stop=(ci == CCH - 1))
                        nc.vector.tensor_copy(
                            dst[:, h, sl * SLAB:(sl + 1) * SLAB], ps[:])
                # v token-major: out [tok 128, HPC*D]
                for half in range(SLAB // 128):
                    ps = psB.tile([128, HPC * D], f32, tag="psB")
                    for ci in range(CCH):
                        nc.tensor.matmul(
                            ps[:],
                            lhsT=_mm(slab[:, ci, half * 128:(half + 1) * 128]),
                            rhs=_mm(wv_s[:, ci, :]),
                            start=(ci == 0), stop=(ci == CCH - 1))
                    nc.scalar.copy(v_tm[:, sl * (SLAB // 128) + half, :], ps[:])
            return q_raw, k_raw, v_tm

        def rope_phase(q_raw, k_raw):
            for src in (q_raw, k_raw):
                for h in range(HPC):
                    shuf = shufp.tile([128, T], qknp, tag="shuf")
                    nc.sync.dma_start(shuf[0:64, :], src[64:128, h, :])
                    nc.sync.dma_start(shuf[64:128, :], src[0:64, h, :])
                    # shuf = shuf * sin' (in place), t2 = q * cos, q = shuf + t2
                    nc.vector.tensor_mul(shuf[:], shuf[:], sin_s[:])
                    t2 = t2p.tile([128, T], qknp, tag="t2")
                    nc.vector.tensor_mul(t2[:], src[:, h, :], cos_s[:])
                    nc.vector.tensor_add(src[:, h, :], shuf[:], t2[:])

        def attn_phase(q_raw, k_raw, v_tm):
            yn = ynp.tile([128, HPC, T], YN_DT, tag="yn")
            for h in range(HPC):
                for s in range(NQS):
                    q0 = s * QS
                    ps_y = psY.tile([128, QS], f32, tag="psY")
                    ps_l = psL.tile([1, QS], f32, tag="psL")
                    njc = 4 * s + 4
                    # diagonal k-chunks first (m=0 is full width and starts
                    # the psum accumulation group), then the off-diagonal
                    # chunks; the last off-diagonal (or m-order last when
                    # s==0) carries stop.
                    order = [4 * s + m for m in range(4)] + list(range(4 * s))
                    for idx, j in enumerate(order):
                        mdiag = j - 4 * s
                        col0 = mdiag * 128 if mdiag >= 0 else 0
                        n = QS - col0
                        qg0 = q0 + col0
                        first = idx == 0
                        last = idx == len(order) - 1
                        st = psA.tile([128, QS], f32, tag="psA")
                        nc.tensor.matmul(
                            st[:, col0:],
                            lhsT=_mm(k_raw[:, h, j * 128:(j + 1) * 128]),
                            rhs=_mm(q_raw[:, h, qg0:q0 + QS]),
                            start=True, stop=True)
                        # bias add (strip offset q - 128j) then exp
                        o = qg0 - j * 128
                        pre = prep.tile([128, QS], f32, tag="pre")
                        nc.vector.tensor_add(
                            pre[:, col0:], st[:, col0:], bias_s[:, h, o:o + n])
                        pt = ptp.tile([128, QS], V_DT, tag="pt")
                        nc.scalar.activation(
                            pt[:, col0:], pre[:, col0:], AF.Exp, scale=SCALE)
                        nc.tensor.matmul(
                            ps_y[:, col0:],
                            lhsT=_mm(v_tm[:, j, h * D:(h + 1) * D]),
                            rhs=_mm(pt[:, col0:]),
                            start=first, stop=last)
                        nc.tensor.matmul(
                            ps_l[0:1, col0:],
                            lhsT=_mm(ones_col[:]),
                            rhs=_mm(pt[:, col0:]),
                            start=first, stop=last)
                    # normalize: yn = y * (1/l) broadcast over partitions
                    rec = smallp.tile([1, QS], f32, tag="rec")
                    nc.vector.reciprocal(rec[:], ps_l[:])
                    ps_b = psBC.tile([128, QS], f32, tag="psBC")
                    nc.tensor.matmul(
                        ps_b[:], lhsT=ones_row[:], rhs=rec[:],
                        start=True, stop=True)
                    rec_bc = bcp.tile([128, QS], f32, tag="recbc")
                    nc.scalar.copy(rec_bc[:], ps_b[:])
                    nc.vector.tensor_mul(
                        yn[:, h, q0:q0 + QS], ps_y[:], rec_bc[:])
            return yn

        def oproj_phase(b, yn):
            for tci in range(T // 128):
                ost = ostp.tile([128, C // 2], f32, tag="ost")
                ost2 = ostp.tile([128, C // 2], f32, tag="ost2")
                stages = (ost, ost2)
                for oc in range(C // 512):
                    ps = psB.tile([128, 512], f32, tag="psB")
                    for h in range(HPC):
                        nc.tensor.matmul(
                            ps[:],
                            lhsT=_mm(yn[:, h, tci * 128:(tci + 1) * 128]),
                            rhs=_mm(wo_s[:, h, oc * 512:(oc + 1) * 512]),
                            start=(h == 0), stop=(h == HPC - 1))
                    dst = stages[oc // 2]
                    dcol = (oc % 2) * 512
                    if oc % 2 == 0:
                        nc.scalar.copy(dst[:, dcol:dcol + 512], ps[:])
                    else:
                        nc.vector.tensor_copy(dst[:, dcol:dcol + 512], ps[:])
                nt = b * (T // 128) + tci
                nc.sync.dma_start(outv[nt, :, 0:C // 2], ost[:])
                nc.sync.dma_start(outv[nt, :, C // 2:C], ost2[:])

        # One mosaic of the mixing attention across batches to keep engines busy:
        # b0 qkv/rope/attn, then b1 qkv overlaps b0 oproj.
        q0r, k0r, v0 = qkv_phase(0)
        rope_phase(q0r, k0r)
        yn0 = attn_phase(q0r, k0r, v0)
        q1r, k1r, v1 = qkv_phase(1)
        oproj_phase(0, yn0)
        rope_phase(q1r, k1r)
        yn1 = attn_phase(q1r, k1r, v1)
        oproj_phase(1, yn1)


def _build_bass():
    nc = bass.Bass("TRN2", target_bir_lowering=False, debug=False,
                   num_devices=NCORES)
    t = {}
    t["xT"] = nc.dram_tensor("xT", [C, BT], f32r, kind="ExternalInput").ap()
    t["wqT"] = nc.dram_tensor("wqT", [C, HPC * D], f32r, kind="ExternalInput").ap()
    t["wkT"] = nc.dram_tensor("wkT", [C, HPC * D], f32r, kind="ExternalInput").ap()
    t["wvT"] = nc.dram_tensor("wvT", [C, HPC * D], f32r, kind="ExternalInput").ap()
    t["woT"] = nc.dram_tensor("woT", [HPC, D, C], YN_DT, kind="ExternalInput").ap()
    t["biasS"] = nc.dram_tensor("biasS", [128, HPC, T], TBL_DT,
                                kind="ExternalInput").ap()
    t["cosT"] = nc.dram_tensor("cosT", [D, T], TBL_DT, kind="ExternalInput").ap()
    t["sinTs"] = nc.dram_tensor("sinTs", [D, T], TBL_DT, kind="ExternalInput").ap()
    t["out"] = nc.dram_tensor("out", [BT, C], f32, kind="ExternalOutput").ap()
    with tile.TileContext(nc) as tc:
        _emit(tc, **t)
    return nc


_STATE = {}


def _get_nc():
    if "nc" not in _STATE:
        _STATE["nc"] = _build_bass()
    return _STATE["nc"]


# ----------------------------------------------------------------------------
# Host-side sharding / unsharding
# ----------------------------------------------------------------------------

def make_in_maps(x, w_qkv, w_o, rel_emb):
    x = np.asarray(x, dtype=np.float32)
    w_qkv = np.asarray(w_qkv, dtype=np.float32)
    w_o = np.asarray(w_o, dtype=np.float32)
    rel_emb = np.asarray(rel_emb, dtype=np.float32)

    xT = np.ascontiguousarray(x.reshape(BT, C).T)          # [C, BT]
    cosT, sinT = _rope_tables()
    cosT = cosT.astype(_np_of(TBL_DT))
    sinT = sinT.astype(_np_of(TBL_DT))
    buck, maskneg = _bucket_strip()

    in_maps = []
    for g in range(NCORES):
        r0 = g * HPC * D
        r1 = (g + 1) * HPC * D
        wq = np.ascontiguousarray(w_qkv[r0:r1, :].T)       # [C, HPC*D]
        wk = np.ascontiguousarray(w_qkv[C + r0:C + r1, :].T)
        wv = np.ascontiguousarray(w_qkv[2 * C + r0:2 * C + r1, :].T)
        wo = np.ascontiguousarray(
            w_o[:, r0:r1].T.reshape(HPC, D, C)).astype(_np_of(YN_DT))
        strips = np.empty((128, HPC, T), dtype=np.float32)
        for h in range(HPC):
            emb_h = rel_emb[:, g * HPC + h]
            sh = emb_h[buck] * np.float32(np.sqrt(D))      # undone by exp scale
            strips[:, h, :] = np.where(maskneg, NEG, sh)
        in_maps.append({
            "xT": xT,
            "wqT": wq, "wkT": wk, "wvT": wv,
            "woT": wo,
            "biasS": strips.astype(_np_of(TBL_DT)),
            "cosT": cosT, "sinTs": sinT,
        })
    return in_maps


def gather_out(results):
    out = np.zeros((BT, C), dtype=np.float32)
    for r in results:
        out += r["out"]
    return out.reshape(B, T, C)


def kernel(**inputs) -> np.ndarray:
    nc = _get_nc()
    in_maps = make_in_maps(inputs["x"], inputs["w_qkv"], inputs["w_o"],
                           inputs["rel_emb"])
    res = run_bass_kernel_spmd(nc, in_maps, core_ids=list(range(NCORES)))
    return gather_out(res.results)


# revision 30
# speedup vs baseline: 17.0647x; 17.0647x over previous
"""Head-sharded (tensor-parallel) causal self-attention with RoPE and T5-style
relative position bias, running SPMD on 8 Trainium2 NeuronCores.

Sharding: heads are split 2-per-core (H=16 over 8 cores). Each core computes
its heads' QKV projection (full token range, fp32r matmuls), RoPE, causal
attention with the rel-pos bias folded into a Toeplitz "bias strip", and a
partial output projection against its slice of w_o's columns. The host sums
the 8 partial outputs.

Attention uses the S^T layout (keys on partitions, queries on the free dim):
  S^T[k, q] = sum_d k'[d,k] q'[d,q]      (PE, contraction over d=128)
  P = exp(S^T/sqrt(D) + bias)            (DVE bias add + ACT exp)
  y^T[d, q] += V^T-free matmul           (PE, lhsT = V token-major)
  l[q] += ones^T @ P                     (PE)
Scores here are O(+-5), so softmax runs without the running-max rescale.
The causal mask and the rel-pos bias are both baked into one per-head
[128, T] strip (bias of block-diagonal offset delta depends only on q-k).
"""

import json
import numpy as np
from contextlib import ExitStack

import concourse.bass as bass
import concourse.bass2jax as _b2j
import concourse.bass_utils as _bu
import concourse.mybir as mybir
import concourse.tile as tile
from concourse.bass_utils import run_bass_kernel_spmd
from concourse.vector_clock import ScopedClock

import ml_dtypes

# ----------------------------------------------------------------------------
# Walrus compatibility: this image's walrus codegen accepts at most ONE sync
# wait per instruction (setupSyncWait "Too many sync wait commands"). Tile
# freely emits multi-wait instructions, so rewrite the BIR before compiling:
# excess waits move onto single-wait NoOps inserted just before the
# instruction on the same engine (same-engine program order makes this
# semantically identical).
# ----------------------------------------------------------------------------

_ws_counter = [0]


def _split_multi_waits(obj):
    if isinstance(obj, dict):
        insts = obj.get("instructions")
        if isinstance(insts, list):
            newl = []
            for ins in insts:
                si = ins.get("sync_info") if isinstance(ins, dict) else None
                waits = (si or {}).get("on_wait") or []
                if len(waits) > 1:
                    for w in waits[:-1]:
                        _ws_counter[0] += 1
                        newl.append({
                            "name": f"I-wsplit-{_ws_counter[0]}",
                            "opcode": "NoOp",
                            "engine": ins["engine"],
                            "ins": [], "outs": [],
                            "debug": ins.get("debug", 0),
                            "sync_info": {"on_wait": [w], "on_update": []},
                        })
                    si["on_wait"] = waits[-1:]
                newl.append(ins)
            obj["instructions"] = newl
        for v in obj.values():
            _split_multi_waits(v)
    elif isinstance(obj, list):
        for v in obj:
            _split_multi_waits(v)


_orig_compile_bir_kernel = _bu.compile_bir_kernel


def _compile_bir_kernel_split_waits(bir_json, *args, **kwargs):
    if isinstance(bir_json, (bytes, bytearray)):
        bir_json = bytes(bir_json)
    bir = json.loads(bir_json)
    _split_multi_waits(bir)
    return _orig_compile_bir_kernel(json.dumps(bir).encode(), *args, **kwargs)


if _bu.compile_bir_kernel is _orig_compile_bir_kernel:
    _bu.compile_bir_kernel = _compile_bir_kernel_split_waits
    _b2j.compile_bir_kernel = _compile_bir_kernel_split_waits


def _drain_and_barrier_split_waits(self, tick_clock, wait_clock):
    """TileContext tail-drain fix for this walrus build: the CoreV3 codegen
    rejects >1 sync-wait on the drain ("Too many sync wait commands"), so park
    the global-clock waits on one NOP each before a bare drain."""
    nop1 = self.nc.sync.nop(nofuse=True)
    wait_clock.add_sem_waits(nop1.ins, ScopedClock({None: tick_clock.global_clock}))
    si = nop1.ins.sync_info
    waits = list(si.on_wait) if si else []
    if len(waits) > 1:
        si.on_wait[:] = waits[:1]
        for w in waits[1:]:
            extra = self.nc.sync.nop(nofuse=True)
            extra.ins.sync_info = mybir.SyncInfo(on_wait=[w], on_update=[])
    self.nc.sync.drain()
    self.nc.all_engine_barrier()
    assert self.sems is not None
    popped = self.nc._tile_sem_poison_stack.pop()
    assert popped is self._sem_poison
    self.nc.clear_and_free_semaphores(list(self.sems.allocated().values()))
    self.nc.all_engine_barrier()


tile.TileContext._drain_and_barrier = _drain_and_barrier_split_waits

# Problem shapes (hardcoded for this task).
B, T, C, H, D = 2, 2048, 2048, 16, 128
NUM_BUCKETS, MAX_DIST = 32, 128
NCORES = 8
HPC = H // NCORES          # heads per core = 2
BT = B * T                 # 4096 tokens
CCH = C // 128             # 16 contraction chunks over C
SLAB = 256                 # qkv token slab width (moving dim, >=256 for f32r rate)
QS = 512                   # attention query subchunk
NQS = T // QS              # 4
NKC = T // 128             # 16 key chunks per sequence
NEG = np.float32(-1.0e30)
SCALE = float(1.0 / np.sqrt(D))

f32 = mybir.dt.float32
f32r = mybir.dt.float32r
bf16 = mybir.dt.bfloat16
AF = mybir.ActivationFunctionType
BF16NP = ml_dtypes.bfloat16

# dtype knobs (accuracy / SBUF tradeoffs)
QK_DT = bf16     # q/k staging + S matmul
V_DT = bf16      # v staging, P tiles, PV matmul dtype
YN_DT = bf16     # normalized attention out + w_o operand dtype
TBL_DT = bf16    # bias strips + rope cos/sin tables


def _np_of(dt):
    return {f32: np.float32, f32r: np.float32, bf16: BF16NP}[dt]


def _mm(ap):
    """Bitcast f32 matmul operands to f32r (full PE rate at N>=256)."""
    return ap.bitcast(f32r) if ap.dtype == f32 else ap


# ----------------------------------------------------------------------------
# Host-side tables (shape-only, cached)
# ----------------------------------------------------------------------------

def _rel_bucket(n):
    """T5 unidirectional bucket of past-distance n (>=0), mirrors reference."""
    me = NUM_BUCKETS // 2
    nf = np.maximum(n.astype(np.float32), np.float32(1.0))
    val = np.log(nf / me).astype(np.float32) / np.float32(np.log(MAX_DIST / me))
    val = (val * np.float32(NUM_BUCKETS - me)).astype(np.float32)
    val_large = me + val.astype(np.int32)
    val_large = np.minimum(val_large, NUM_BUCKETS - 1)
    return np.where(n < me, n, val_large)


_TBL_CACHE = {}


def _bucket_strip():
    """[128, T] int bucket ids for strip position (q' - kk); -1 where masked."""
    if "bstrip" not in _TBL_CACHE:
        kk = np.arange(128)[:, None]
        qp = np.arange(T)[None, :]
        n = qp - kk                              # distance into the past
        buck = _rel_bucket(np.maximum(n, 0))
        _TBL_CACHE["bstrip"] = (buck, n < 0)
    return _TBL_CACHE["bstrip"]


def _rope_tables():
    if "rope" not in _TBL_CACHE:
        half = np.arange(0, D, 2, dtype=np.float32) / np.float32(D)
        inv_freq = (1.0 / (10000.0 ** half)).astype(np.float32)
        freqs = np.arange(T, dtype=np.float32)[:, None] * inv_freq[None, :]
        emb = np.concatenate([freqs, freqs], axis=1).astype(np.float32)  # [T, D]
        cosT = np.ascontiguousarray(np.cos(emb).T)     # [D, T]
        sinT = np.ascontiguousarray(np.sin(emb).T)
        sinT[: D // 2] = -sinT[: D // 2]               # fold rotate_half sign
        _TBL_CACHE["rope"] = (cosT, sinT)
    return _TBL_CACHE["rope"]


# ----------------------------------------------------------------------------
# Device program
# ----------------------------------------------------------------------------

def _emit(tc, xT, wqT, wkT, wvT, woT, biasS, cosT, sinTs, out):
    nc = tc.nc
    qknp = QK_DT

    xTv = xT.rearrange("(i p) n -> p i n", p=128)       # [128, CCH, BT]
    wqv = wqT.rearrange("(i p) f -> p i f", p=128)      # [128, CCH, HPC*D]
    wkv = wkT.rearrange("(i p) f -> p i f", p=128)
    wvv = wvT.rearrange("(i p) f -> p i f", p=128)
    wov = woT.rearrange("h d n -> d h n")               # [128, HPC, C]
    outv = out.rearrange("(nt p) c -> nt p c", p=128)   # [BT/128, 128, C]

    with ExitStack() as ctx:
        ep = ctx.enter_context

        # SBUF pools
        wpool = ep(tc.tile_pool(name="weights", bufs=1))
        xpool = ep(tc.tile_pool(name="xslab", bufs=2))
        qkvp = ep(tc.tile_pool(name="qkv", bufs=1))
        shufp = ep(tc.tile_pool(name="shuf", bufs=2))
        t2p = ep(tc.tile_pool(name="t2", bufs=2))
        prep = ep(tc.tile_pool(name="pre", bufs=4))
        ptp = ep(tc.tile_pool(name="pt", bufs=6))
        ynp = ep(tc.tile_pool(name="yn", bufs=1))
        ostp = ep(tc.tile_pool(name="ost", bufs=2))
        smallp = ep(tc.tile_pool(name="small", bufs=2))
        bcp = ep(tc.tile_pool(name="bcp", bufs=1))

        # PSUM pools (8 banks total budget)
        psA = ep(tc.tile_pool(name="psA", bufs=3, space="PSUM"))   # qk-proj + S
        psB = ep(tc.tile_pool(name="psB", bufs=2, space="PSUM"))   # v-proj + oproj
        psY = ep(tc.tile_pool(name="psY", bufs=1, space="PSUM"))   # y^T accum
        psL = ep(tc.tile_pool(name="psL", bufs=1, space="PSUM"))   # l accum
        psBC = ep(tc.tile_pool(name="psBC", bufs=1, space="PSUM"))  # 1/l broadcast

        # Resident weights/tables. Chunk and order the DMA stream by first
        # use: w chunk 0 + the first x slab unblock the first matmuls within
        # a few us; attention tables (bias/wo) ride behind batch-0 QKV.
        wq_s = wpool.tile([128, CCH, HPC * D], bf16, tag="wq")
        wk_s = wpool.tile([128, CCH, HPC * D], bf16, tag="wk")
        wv_s = wpool.tile([128, CCH, HPC * D], bf16, tag="wv")
        cs = slice(0, 4)
        nc.sync.dma_start(wq_s[:, cs, :], wqv[:, cs, :])
        nc.sync.dma_start(wk_s[:, cs, :], wkv[:, cs, :])
        nc.sync.dma_start(wv_s[:, cs, :], wvv[:, cs, :])
        slab0 = xpool.tile([128, CCH, SLAB], bf16, tag="slab")
        nc.sync.dma_start(slab0[:], xTv[:, :, 0:SLAB])
        for ci4 in range(1, CCH // 4):
            cs = slice(4 * ci4, 4 * ci4 + 4)
            nc.sync.dma_start(wq_s[:, cs, :], wqv[:, cs, :])
            nc.sync.dma_start(wk_s[:, cs, :], wkv[:, cs, :])
            nc.sync.dma_start(wv_s[:, cs, :], wvv[:, cs, :])
        cos_s = wpool.tile([128, T], TBL_DT, tag="cos")
        nc.sync.dma_start(cos_s[:], cosT[:])
        sin_s = wpool.tile([128, T], TBL_DT, tag="sin")
        nc.sync.dma_start(sin_s[:], sinTs[:])
        bias_s = wpool.tile([128, HPC, T], TBL_DT, tag="bias")
        wo_s = wpool.tile([128, HPC, C], YN_DT, tag="wo")
        ones_col = wpool.tile([128, 1], V_DT, tag="ones_col")
        nc.gpsimd.memset(ones_col[:], 1.0)
        ones_row_f = wpool.tile([1, 128], f32, tag="ones_row_f")
        nc.gpsimd.memset(ones_row_f[:], 1.0)
        ones_row_r = wpool.tile([1, 128], f32r, tag="ones_row")
        nc.scalar.copy(ones_row_r[:], ones_row_f[:])

        def qkv_phase(b, first_slab=None):
            q_raw = qkvp.tile([128, HPC, T], qknp, tag="qraw")
            k_raw = qkvp.tile([128, HPC, T], qknp, tag="kraw")
            v_tm = qkvp.tile([128, NKC, HPC * D], V_DT, tag="vtm")
            for sl in range(T // SLAB):
                t0 = b * T + sl * SLAB
                tl = sl * SLAB                     # batch-local token offset
                if sl == 0 and first_slab is not None:
                    slab = first_slab
                else:
                    slab = xpool.tile([128, CCH, SLAB], bf16, tag="slab")
                    nc.sync.dma_start(slab[:], xTv[:, :, t0:t0 + SLAB])
                if sl == 2 and b == 0:
                    # attention tables: queued once the pipeline is rolling
                    nc.sync.dma_start(bias_s[:], biasS[:])
                    nc.sync.dma_start(wo_s[:], wov[:])
                # q/k feature-major: out [feat 128, tok SLAB], rope fused
                for wt, dst, nm in ((wq_s, q_raw, "q"), (wk_s, k_raw, "k")):
                    for h in range(HPC):
                        ps = psA.tile([128, SLAB], f32, tag="psA")
                        for ci in range(CCH):
                            nc.tensor.matmul(
                                ps[:],
                                lhsT=_mm(wt[:, ci, h * D:(h + 1) * D]),
                                rhs=_mm(slab[:, ci, :]),
                                start=(ci == 0), stop=(ci == CCH - 1))
                        dstv = dst[:, h, tl:tl + SLAB]
                        if h == 0:
                            nc.vector.tensor_copy(dstv, ps[:])
                        else:
                            nc.scalar.copy(dstv, ps[:])
                        # rope: dst = dst*cos + halfswap(dst)*sin'
                        shuf = shufp.tile([128, SLAB], qknp, tag="shuf")
                        nc.sync.dma_start(shuf[0:64, :], dst[64:128, h, tl:tl + SLAB])
                        nc.sync.dma_start(shuf[64:128, :], dst[0:64, h, tl:tl + SLAB])
                        nc.vector.tensor_mul(
                            shuf[:], shuf[:], sin_s[:, tl:tl + SLAB])
                        t2 = t2p.tile([128, SLAB], qknp, tag="t2")
                        nc.vector.tensor_mul(
                            t2[:], dstv, cos_s[:, tl:tl + SLAB])
                        nc.vector.tensor_add(dstv, shuf[:], t2[:])
                # v token-major: out [tok 128, HPC*D]
                for half in range(SLAB // 128):
                    ps = psB.tile([128, HPC * D], f32, tag="psB")
                    for ci in range(CCH):
                        nc.tensor.matmul(
                            ps[:],
                            lhsT=_mm(slab[:, ci, half * 128:(half + 1) * 128]),
                            rhs=_mm(wv_s[:, ci, :]),
                            start=(ci == 0), stop=(ci == CCH - 1))
                    nc.scalar.copy(v_tm[:, sl * (SLAB // 128) + half, :], ps[:])
            return q_raw, k_raw, v_tm

        def attn_phase(q_raw, k_raw, v_tm):
            yn = ynp.tile([128, HPC, T], YN_DT, tag="yn")
            for h in range(HPC):
                for s in range(NQS):
                    q0 = s * QS
                    ps_y = psY.tile([128, QS], f32, tag="psY")
                    ps_l = psL.tile([1, QS], f32, tag="psL")
                    njc = 4 * s + 4
                    # diagonal k-chunks first (m=0 is full width and starts
                    # the psum accumulation group), then the off-diagonal
                    # chunks; the last off-diagonal (or m-order last when
                    # s==0) carries stop.
                    order = [4 * s + m for m in range(4)] + list(range(4 * s))
                    for idx, j in enumerate(order):
                        mdiag = j - 4 * s
                        col0 = mdiag * 128 if mdiag >= 0 else 0
                        n = QS - col0
                        qg0 = q0 + col0
                        first = idx == 0
                        last = idx == len(order) - 1
                        st = psA.tile([128, QS], f32, tag="psA")
                        nc.tensor.matmul(
                            st[:, col0:],
                            lhsT=_mm(k_raw[:, h, j * 128:(j + 1) * 128]),
                            rhs=_mm(q_raw[:, h, qg0:q0 + QS]),
                            start=True, stop=True)
                        # bias add (strip offset q - 128j) then exp
                        o = qg0 - j * 128
                        pre = prep.tile([128, QS], f32, tag="pre")
                        nc.vector.tensor_add(
                            pre[:, col0:], st[:, col0:], bias_s[:, h, o:o + n])
                        pt = ptp.tile([128, QS], V_DT, tag="pt")
                        nc.scalar.activation(
                            pt[:, col0:], pre[:, col0:], AF.Exp, scale=SCALE)
                        nc.tensor.matmul(
                            ps_y[:, col0:],
                            lhsT=_mm(v_tm[:, j, h * D:(h + 1) * D]),
                            rhs=_mm(pt[:, col0:]),
                            start=first, stop=last)
                        nc.tensor.matmul(
                            ps_l[0:1, col0:],
                            lhsT=_mm(ones_col[:]),
                            rhs=_mm(pt[:, col0:]),
                            start=first, stop=last)
                    # normalize: yn = y * (1/l) broadcast over partitions
                    rec = smallp.tile([1, QS], f32, tag="rec")
                    nc.vector.reciprocal(rec[:], ps_l[:])
                    rec_r = smallp.tile([1, QS], f32r, tag="recr")
                    nc.scalar.copy(rec_r[:], rec[:])
                    ps_b = psBC.tile([128, QS], f32, tag="psBC")
                    nc.tensor.matmul(
                        ps_b[:], lhsT=ones_row_r[:], rhs=rec_r[:],
                        start=True, stop=True)
                    rec_bc = bcp.tile([128, QS], f32, tag="recbc")
                    nc.scalar.copy(rec_bc[:], ps_b[:])
                    nc.vector.tensor_mul(
                        yn[:, h, q0:q0 + QS], ps_y[:], rec_bc[:])
            return yn

        def oproj_phase(b, yn):
            for tci in range(T // 128):
                ost = ostp.tile([128, C // 2], f32, tag="ost")
                ost2 = ostp.tile([128, C // 2], f32, tag="ost2")
                stages = (ost, ost2)
                for oc in range(C // 512):
                    ps = psB.tile([128, 512], f32, tag="psB")
                    for h in range(HPC):
                        nc.tensor.matmul(
                            ps[:],
                            lhsT=_mm(yn[:, h, tci * 128:(tci + 1) * 128]),
                            rhs=_mm(wo_s[:, h, oc * 512:(oc + 1) * 512]),
                            start=(h == 0), stop=(h == HPC - 1))
                    dst = stages[oc // 2]
                    dcol = (oc % 2) * 512
                    if oc % 2 == 0:
                        nc.scalar.copy(dst[:, dcol:dcol + 512], ps[:])
                    else:
                        nc.vector.tensor_copy(dst[:, dcol:dcol + 512], ps[:])
                nt = b * (T // 128) + tci
                nc.sync.dma_start(outv[nt, :, 0:C // 2], ost[:])
                nc.sync.dma_start(outv[nt, :, C // 2:C], ost2[:])

        # Interleave batches to keep engines busy: b1 qkv overlaps b0 oproj.
        import os
        phases = os.environ.get("KPHASES", "all")
        if phases == "qkv":
            qkv_phase(0, first_slab=slab0)
            qkv_phase(1)
        elif phases == "attn":
            q0r, k0r, v0 = qkv_phase(0, first_slab=slab0)
            attn_phase(q0r, k0r, v0)
            q1r, k1r, v1 = qkv_phase(1)
            attn_phase(q1r, k1r, v1)
        else:
            q0r, k0r, v0 = qkv_phase(0, first_slab=slab0)
            yn0 = attn_phase(q0r, k0r, v0)
            q1r, k1r, v1 = qkv_phase(1)
            oproj_phase(0, yn0)
            yn1 = attn_phase(q1r, k1r, v1)
            oproj_phase(1, yn1)


def _build_bass():
    nc = bass.Bass("TRN2", target_bir_lowering=False, debug=False,
                   num_devices=NCORES)
    t = {}
    t["xT"] = nc.dram_tensor("xT", [C, BT], bf16, kind="ExternalInput").ap()
    t["wqT"] = nc.dram_tensor("wqT", [C, HPC * D], bf16, kind="ExternalInput").ap()
    t["wkT"] = nc.dram_tensor("wkT", [C, HPC * D], bf16, kind="ExternalInput").ap()
    t["wvT"] = nc.dram_tensor("wvT", [C, HPC * D], bf16, kind="ExternalInput").ap()
    t["woT"] = nc.dram_tensor("woT", [HPC, D, C], YN_DT, kind="ExternalInput").ap()
    t["biasS"] = nc.dram_tensor("biasS", [128, HPC, T], TBL_DT,
                                kind="ExternalInput").ap()
    t["cosT"] = nc.dram_tensor("cosT", [D, T], TBL_DT, kind="ExternalInput").ap()
    t["sinTs"] = nc.dram_tensor("sinTs", [D, T], TBL_DT, kind="ExternalInput").ap()
    t["out"] = nc.dram_tensor("out", [BT, C], f32, kind="ExternalOutput").ap()
    with tile.TileContext(nc) as tc:
        _emit(tc, **t)
    return nc


_STATE = {}


def _get_nc():
    if "nc" not in _STATE:
        _STATE["nc"] = _build_bass()
    return _STATE["nc"]


# ----------------------------------------------------------------------------
# Host-side sharding / unsharding
# ----------------------------------------------------------------------------

def make_in_maps(x, w_qkv, w_o, rel_emb):
    x = np.asarray(x, dtype=np.float32)
    w_qkv = np.asarray(w_qkv, dtype=np.float32)
    w_o = np.asarray(w_o, dtype=np.float32)
    rel_emb = np.asarray(rel_emb, dtype=np.float32)

    xT = np.ascontiguousarray(x.reshape(BT, C).T).astype(BF16NP)   # [C, BT]
    cosT, sinT = _rope_tables()
    cosT = cosT.astype(_np_of(TBL_DT))
    sinT = sinT.astype(_np_of(TBL_DT))
    buck, maskneg = _bucket_strip()

    in_maps = []
    for g in range(NCORES):
        r0 = g * HPC * D
        r1 = (g + 1) * HPC * D
        wq = np.ascontiguousarray(w_qkv[r0:r1, :].T).astype(BF16NP)
        wk = np.ascontiguousarray(w_qkv[C + r0:C + r1, :].T).astype(BF16NP)
        wv = np.ascontiguousarray(w_qkv[2 * C + r0:2 * C + r1, :].T).astype(BF16NP)
        wo = np.ascontiguousarray(
            w_o[:, r0:r1].T.reshape(HPC, D, C)).astype(_np_of(YN_DT))
        strips = np.empty((128, HPC, T), dtype=np.float32)
        for h in range(HPC):
            emb_h = rel_emb[:, g * HPC + h]
            sh = emb_h[buck] * np.float32(np.sqrt(D))      # undone by exp scale
            strips[:, h, :] = np.where(maskneg, NEG, sh)
        in_maps.append({
            "xT": xT,
            "wqT": wq, "wkT": wk, "wvT": wv,
            "woT": wo,
            "biasS": strips.astype(_np_of(TBL_DT)),
            "cosT": cosT, "sinTs": sinT,
        })
    return in_maps


def gather_out(results):
    out = np.zeros((BT, C), dtype=np.float32)
    for r in results:
        out += r["out"]
    return out.reshape(B, T, C)


def kernel(**inputs) -> np.ndarray:
    nc = _get_nc()
    in_maps = make_in_maps(inputs["x"], inputs["w_qkv"], inputs["w_o"],
                           inputs["rel_emb"])
    res = run_bass_kernel_spmd(nc, in_maps, core_ids=list(range(NCORES)))
    return gather_out(res.results)
